# revision 17
# baseline (speedup 1.0000x reference)
"""TRN2 Bass kernel for ChemicalConvWithBonds GNN message passing.

h_out = segment_sum(silu([h[row] | h[col] | bond] @ W1 + b1) @ W2 + b2, row)

Strategy (8 NeuronCores, SPMD):
  - Edges are owner-sharded by destination node (row): core c owns nodes
    [c*6250, (c+1)*6250). The host sorts edges by row and pads each
    128-node window to a tile count shared across all cores (one SPMD graph).
  - Device precomputes P = h@W1a + b1 and Q = h@W1b for its node slice
    (bf16), then AllGathers Q so every core holds the full Q table.
  - Per 128-edge tile: Q[col] rows fetched by indirect DMA; P[row] applied
    with a one-hot matmul from the window's P block (no gather); bond@W1c
    by matmul; all summed in PSUM; SiLU on the scalar engine; scatter-add
    into S^T via a one-hot matmul; per window O^T = W2-matmul + b2 x deg.
"""

import os
import sys

for p in ("/opt/trn_rl_repo", "/root/.axon_site/_ro/trn_rl_repo"):
    if os.path.isdir(p) and p not in sys.path:
        sys.path.append(p)

import contextlib
import ctypes
import types

import numpy as np
import ml_dtypes

import concourse.tile as tile
from concourse import mybir
from concourse.bacc import Bacc
from concourse.bass import IndirectOffsetOnAxis
from concourse.tile_rust import add_dep_helper

C = 8
H = 128
BOND = 16
G = 4
NPC = 6250
SCRATCH = 57344

bf = mybir.dt.bfloat16
f32 = mybir.dt.float32
i32 = mybir.dt.int32
nbf = ml_dtypes.bfloat16

LAST_EXEC_NS = None


def _install_ntff_hook():
    """Provide antenv.axon_hooks so trace=True works under axon."""
    if "antenv.axon_hooks" in sys.modules:
        return
    so_path = "/opt/axon/libaxon_pjrt.so"
    try:
        lib = ctypes.CDLL(so_path)
        lib.axon_start_nrt_profile.argtypes = [
            ctypes.POINTER(ctypes.c_int64),
            ctypes.c_size_t,
        ]
        lib.axon_start_nrt_profile.restype = ctypes.c_int64
        lib.axon_stop_nrt_profile.argtypes = [ctypes.c_char_p]
        lib.axon_stop_nrt_profile.restype = ctypes.c_int64
    except Exception:
        return

    @contextlib.contextmanager
    def _hook(output_dir, device_ids):
        import jax

        jax.devices()
        if device_ids:
            ids = (ctypes.c_int64 * len(device_ids))(*device_ids)
            rc = lib.axon_start_nrt_profile(ids, len(device_ids))
        else:
            rc = lib.axon_start_nrt_profile(None, 0)
        if rc != 0:
            raise RuntimeError(f"axon_start_nrt_profile rc={rc}")
        try:
            yield
        finally:
            n = lib.axon_stop_nrt_profile(str(output_dir).encode())
            print(f"profile: {n} file(s) -> {output_dir}", file=sys.stderr)

    mod = types.ModuleType("antenv.axon_hooks")
    mod.get_axon_ntff_profile_hook = lambda: _hook
    mod.set_axon_ntff_profile_hook = lambda h: None
    sys.modules["antenv.axon_hooks"] = mod


def host_prep(h, edge_index, bond_features, W1, b1, W2, b2, npc=NPC):
    N, _ = h.shape
    W = (npc + 127) // 128
    npad = W * 128
    nhalf = (C // 2) * npad  # Q table split point (int16 index limit)
    QSUB = 1024  # edges per dma_gather call

    row = np.asarray(edge_index[0], dtype=np.int64)
    col = np.asarray(edge_index[1], dtype=np.int64)
    perm = np.argsort(row, kind="stable")
    rs = row[perm]
    cs = col[perm]
    bs = np.asarray(bond_features, dtype=np.float32)[perm]
    qp = (cs // npc) * npad + cs % npc  # padded Q row index

    cb = np.searchsorted(rs, np.arange(C + 1) * npc)
    wb = np.zeros((C, W + 1), dtype=np.int64)
    for c in range(C):
        lo, hi = cb[c], cb[c + 1]
        wb[c] = lo + np.searchsorted(rs[lo:hi], c * npc + np.arange(W + 1) * 128)

    # per (core, window): split by Q half; pad each half to shared tile counts
    cntA = np.zeros((C, W), np.int64)
    cntB = np.zeros((C, W), np.int64)
    for c in range(C):
        for w in range(W):
            lo, hi = wb[c, w], wb[c, w + 1]
            hv = qp[lo:hi] >= nhalf
            cntB[c, w] = hv.sum()
            cntA[c, w] = (hi - lo) - cntB[c, w]
    NIa = (-(-cntA.max(axis=0) // 128) * 128).astype(np.int64)
    NIb = (-(-cntB.max(axis=0) // 128) * 128).astype(np.int64)
    Tw = ((NIa + NIb) // 128).astype(np.int64)
    Tw = np.maximum(Tw, 1)
    NIa = np.where(NIa + NIb == 0, 128, NIa)
    NT = int(Tw.sum())
    t0s = np.concatenate([[0], np.cumsum(Tw)])

    lrow = np.full((C, NT * 128), 255.0, np.float32)
    bondT = np.zeros((C, BOND, NT * 128), np.float32)
    degT = np.zeros((C, 1, npad), np.float32)
    q16 = np.zeros((C, 128, NT * 8), np.int16)  # wrapped+replicated gather idx
    p16 = np.zeros((C, 128, NT * 8), np.int16)

    def wrap16(dst, base_col, vals):
        # idx i of this call at [i%16 (+16g), base_col + i//16]
        n = len(vals)
        blk = vals.reshape(n // 16, 16).T.astype(np.int16)
        for g in range(8):
            dst[g * 16 : (g + 1) * 16, base_col : base_col + n // 16] = blk

    for c in range(C):
        deg = np.bincount(rs[cb[c] : cb[c + 1]] - c * npc, minlength=npad)
        degT[c, 0, :] = deg[:npad]
        for w in range(W):
            lo, hi = wb[c, w], wb[c, w + 1]
            qv = qp[lo:hi]
            hv = qv >= nhalf
            order = np.argsort(hv, kind="stable")
            na, nb = int(cntA[c, w]), int(cntB[c, w])
            s = int(t0s[w]) * 128
            # half A occupies slots [0, NIa), half B [NIa, NIa+NIb)
            posA = s + np.arange(na)
            posB = s + int(NIa[w]) + np.arange(nb)
            pos = np.concatenate([posA, posB])
            src_idx = lo + order
            lrow[c][pos] = rs[src_idx] - c * npc - w * 128
            bondT[c][:, pos] = bs[src_idx].T
            qa = np.zeros(int(NIa[w]), np.int64)
            qa[:na] = qv[order[:na]]
            qb = np.zeros(int(NIb[w]), np.int64)
            qb[:nb] = qv[order[na:]] - nhalf
            # wrap per QSUB-sized call
            for half, arr in ((0, qa), (1, qb)):
                off = 0
                base = s if half == 0 else s + int(NIa[w])
                while off < len(arr):
                    ni = min(QSUB, len(arr) - off)
                    wrap16(q16[c], (base + off) // 16, arr[off : off + ni])
                    off += ni
            # P gather indices: local row per slot (pads -> 0)
            pv = np.zeros(int(Tw[w]) * 128, np.int64)
            pv[pos - s] = rs[src_idx] - c * npc
            off = 0
            while off < len(pv):
                ni = min(QSUB, len(pv) - off)
                wrap16(p16[c], (s + off) // 16, pv[off : off + ni])
                off += ni

    W1 = np.asarray(W1, np.float32)
    W1ab = np.ascontiguousarray(np.concatenate([W1[:H], W1[H : 2 * H]], axis=1))
    W1cT = np.asarray(W1[2 * H :], np.float32).astype(nbf)
    b1z = np.concatenate([np.asarray(b1, np.float32), np.zeros(H, np.float32)])[None]
    hT_all = np.ascontiguousarray(np.asarray(h, np.float32).T)

    shared = {
        "W1ab": W1ab,
        "b1z": b1z,
        "onesr": np.ones((1, H), np.float32),
        "W1cT": W1cT,
        "W2p": np.asarray(W2, np.float32).astype(nbf),
        "b2r": np.asarray(b2, np.float32)[None].astype(nbf),
        "ident": np.eye(H, dtype=np.float32).astype(nbf),
        "iotag": np.tile(np.arange(128, dtype=np.float32), (128, G)).astype(nbf),
    }
    in_maps = []
    for c in range(C):
        hT_c = np.zeros((H, npad), np.float32)
        hT_c[:, :npc] = hT_all[:, c * npc : (c + 1) * npc]
        m = dict(shared)
        m["hT"] = hT_c
        m["q16"] = q16[c]
        m["p16"] = p16[c]
        m["lrowp"] = np.ascontiguousarray(
            lrow[c].reshape(NT, 128).T
        ).astype(nbf)
        m["bondT"] = bondT[c].astype(nbf)
        m["degT"] = degT[c].astype(nbf)
        in_maps.append(m)

    meta = {
        "npc": npc,
        "npad": npad,
        "W": W,
        "Tw": [int(x) for x in Tw],
        "NIa": [int(x) for x in NIa],
        "NIb": [int(x) for x in NIb],
        "NT": NT,
        "QSUB": QSUB,
    }
    return meta, in_maps


def build(meta):
    npad = meta["npad"]
    W = meta["W"]
    Tw = meta["Tw"]
    NIa = meta["NIa"]
    NIb = meta["NIb"]
    NT = meta["NT"]
    QSUB = meta["QSUB"]
    Twmax = max(Tw)
    nhalf = (C // 2) * npad

    nc = Bacc(dynamic_dma_scratch_size=SCRATCH, num_swdge_queues=4)
    hT = nc.declare_dram_parameter("hT", [H, npad], f32, isOutput=False)
    W1ab = nc.declare_dram_parameter("W1ab", [H, 2 * H], f32, isOutput=False)
    b1z = nc.declare_dram_parameter("b1z", [1, 2 * H], f32, isOutput=False)
    onesr = nc.declare_dram_parameter("onesr", [1, H], f32, isOutput=False)
    W1cT = nc.declare_dram_parameter("W1cT", [BOND, H], bf, isOutput=False)
    W2p = nc.declare_dram_parameter("W2p", [H, H], bf, isOutput=False)
    b2r = nc.declare_dram_parameter("b2r", [1, H], bf, isOutput=False)
    ident = nc.declare_dram_parameter("ident", [H, H], bf, isOutput=False)
    iotag = nc.declare_dram_parameter("iotag", [128, G * 128], bf, isOutput=False)
    q16 = nc.declare_dram_parameter("q16", [128, NT * 8], mybir.dt.int16, isOutput=False)
    p16 = nc.declare_dram_parameter("p16", [128, NT * 8], mybir.dt.int16, isOutput=False)
    lrowp = nc.declare_dram_parameter("lrowp", [128, NT], bf, isOutput=False)
    bondT = nc.declare_dram_parameter("bondT", [BOND, NT * 128], bf, isOutput=False)
    degT = nc.declare_dram_parameter("degT", [1, npad], bf, isOutput=False)
    outT = nc.declare_dram_parameter("outT", [H, npad], f32, isOutput=True)

    P_loc = nc.dram_tensor("P_loc", [npad, H], bf)
    Q_self = nc.dram_tensor("Q_self", [npad, H], bf)
    Q_full = nc.dram_tensor("Q_full", [C * npad, H], bf, addr_space="Shared")

    SILU = mybir.ActivationFunctionType.Silu

    with tile.TileContext(nc) as tc:
        with tc.tile_pool(name="cst", bufs=1) as cp:
            W1ab_sb = cp.tile([H, 2 * H], f32)
            nc.sync.dma_start(out=W1ab_sb[:], in_=W1ab[:])
            b1z_sb = cp.tile([1, 2 * H], f32)
            nc.sync.dma_start(out=b1z_sb[:], in_=b1z[:])
            ones_sb = cp.tile([1, H], f32)
            nc.sync.dma_start(out=ones_sb[:], in_=onesr[:])
            W1c_sb = cp.tile([BOND, H], bf)
            nc.sync.dma_start(out=W1c_sb[:], in_=W1cT[:])
            W2_sb = cp.tile([H, H], bf)
            nc.sync.dma_start(out=W2_sb[:], in_=W2p[:])
            b2_sb = cp.tile([1, H], bf)
            nc.sync.dma_start(out=b2_sb[:], in_=b2r[:])
            id_sb = cp.tile([H, H], bf)
            nc.sync.dma_start(out=id_sb[:], in_=ident[:])
            iota_sb = cp.tile([128, G * 128], bf)
            nc.sync.dma_start(out=iota_sb[:], in_=iotag[:])
            q16_sb = cp.tile([128, NT * 8], mybir.dt.int16)
            q16_ld = nc.sync.dma_start(out=q16_sb[:], in_=q16[:])
            p16_sb = cp.tile([128, NT * 8], mybir.dt.int16)
            p16_ld = nc.sync.dma_start(out=p16_sb[:], in_=p16[:])
            lrow_sb = cp.tile([128, NT], bf)
            nc.sync.dma_start(out=lrow_sb[:], in_=lrowp[:])
            degT_sb = cp.tile([1, npad], bf)
            nc.sync.dma_start(out=degT_sb[:], in_=degT[:])

            # ---- Phase A ----
            p_writes = []
            pa_ctx = tc.tile_pool(name="pa", bufs=2)
            pap_ctx = tc.tile_pool(name="pap", bufs=2, space="PSUM")
            pa = pa_ctx.__enter__()
            pap = pap_ctx.__enter__()
            hT_sb = pa.tile([H, npad], f32, tag="hTsb", name="hTsb")
            nc.sync.dma_start(out=hT_sb[:], in_=hT[:])
            for w in range(W):
                ppq = pap.tile([128, 2 * H], f32, tag="ppq", name=f"ppq{w}")
                nc.tensor.matmul(
                    ppq[:],
                    lhsT=hT_sb[:, w * 128 : (w + 1) * 128],
                    rhs=W1ab_sb[:],
                    start=True,
                    stop=False,
                )
                nc.tensor.matmul(
                    ppq[:], lhsT=ones_sb[:], rhs=b1z_sb[:], start=False, stop=True
                )
                pq_sb = pa.tile([128, 2 * H], bf, tag="pqsb", name=f"pq{w}")
                nc.scalar.copy(out=pq_sb[:], in_=ppq[:])
                p_writes.append(
                    nc.sync.dma_start(
                        out=P_loc[w * 128 : (w + 1) * 128, :], in_=pq_sb[:, 0:H]
                    )
                )
                nc.sync.dma_start(
                    out=Q_self[w * 128 : (w + 1) * 128, :], in_=pq_sb[:, H : 2 * H]
                )

            cc = nc.gpsimd.collective_compute(
                "AllGather",
                mybir.AluOpType.bypass,
                replica_groups=[list(range(C))],
                ins=[Q_self[:]],
                outs=[Q_full[:]],
            )
            pap_ctx.__exit__(None, None, None)
            pa_ctx.__exit__(None, None, None)

            # ---- Phase B ----
            qnum = 0
            with (
                tc.tile_pool(name="gp", bufs=3) as gp,
                tc.tile_pool(name="bp", bufs=2) as bp,
                tc.tile_pool(name="sp", bufs=4) as sp,
                tc.tile_pool(name="mp", bufs=4) as mp,
                tc.tile_pool(name="zp", bufs=3, space="PSUM") as zp,
                tc.tile_pool(name="Sp", bufs=2, space="PSUM") as Sp,
                tc.tile_pool(name="Op", bufs=1, space="PSUM") as Op,
                tc.tile_pool(name="op", bufs=3) as op,
            ):
                t0 = 0
                for w in range(W):
                    T = Tw[w]
                    bT = bp.tile([BOND, T * 128], bf, tag="bT", name=f"bT{w}")
                    nc.sync.dma_start(
                        out=bT[:], in_=bondT[:, t0 * 128 : (t0 + T) * 128]
                    )
                    # ---- P gathers (local table), rotating queues
                    Pg = gp.tile([128, T * 128], bf, tag="Pg", name=f"Pg{w}")
                    off = 0
                    while off < T * 128:
                        ni = min(QSUB, T * 128 - off)
                        gi = nc.gpsimd.dma_gather(
                            out_ap=Pg[:, off : off + ni].rearrange(
                                "p (k h) -> p k h", h=H
                            ),
                            in_ap=P_loc[:],
                            idxs_ap=p16_sb[
                                :, (t0 * 128 + off) // 16 : (t0 * 128 + off + ni) // 16
                            ],
                            num_idxs=ni,
                            num_idxs_reg=ni,
                            elem_size=H,
                            queue_num=qnum % 4,
                        )
                        qnum += 1
                        add_dep_helper(gi.ins, p_writes[w].ins, sync=True, reason="P win ready")
                        add_dep_helper(gi.ins, p16_ld.ins, sync=True, reason="after idx")
                        off += ni
                    # ---- Q gathers: per half, per QSUB chunk, rotating queues
                    Qg = gp.tile([128, T * 128], bf, tag="Qg", name=f"Qg{w}")
                    for half in range(2):
                        ni_h = NIa[w] if half == 0 else NIb[w]
                        base = 0 if half == 0 else NIa[w]
                        rowbase = 0 if half == 0 else nhalf
                        off = 0
                        while off < ni_h:
                            ni = min(QSUB, ni_h - off)
                            slot = base + off
                            gi = nc.gpsimd.dma_gather(
                                out_ap=Qg[:, slot : slot + ni].rearrange(
                                    "p (k h) -> p k h", h=H
                                ),
                                in_ap=Q_full[rowbase : rowbase + nhalf, :],
                                idxs_ap=q16_sb[
                                    :, (t0 * 128 + slot) // 16 : (t0 * 128 + slot + ni) // 16
                                ],
                                num_idxs=ni,
                                num_idxs_reg=ni,
                                elem_size=H,
                                queue_num=qnum % 4,
                            )
                            qnum += 1
                            add_dep_helper(
                                gi.ins, cc.ins, sync=True, reason="after AllGather"
                            )
                            add_dep_helper(
                                gi.ins, q16_ld.ins, sync=True, reason="after idx"
                            )
                            off += ni
                    pS = Sp.tile([128, 128], f32, tag="pS", name=f"pS{w}")
                    g0 = 0
                    while g0 < T:
                        gs = min(G, T - g0)
                        pz = zp.tile([128, G * 128], f32, tag="pz", name=f"pz{w}_{g0}")
                        for i in range(gs):
                            t = g0 + i
                            sl = slice(i * 128, (i + 1) * 128)
                            esl = slice(t * 128, (t + 1) * 128)
                            nc.tensor.matmul(
                                pz[:, sl],
                                lhsT=bT[:, esl],
                                rhs=W1c_sb[:],
                                start=True,
                                stop=True,
                            )
                        a_sb = sp.tile([128, G * 128], bf, tag="a", name=f"a{w}_{g0}")
                        nc.vector.tensor_tensor(
                            out=a_sb[:, : gs * 128],
                            in0=Pg[:, g0 * 128 : g0 * 128 + gs * 128],
                            in1=Qg[:, g0 * 128 : g0 * 128 + gs * 128],
                            op=mybir.AluOpType.add,
                        )
                        zq_sb = sp.tile(
                            [128, G * 128], f32, tag="zq", name=f"zq{w}_{g0}"
                        )
                        nc.vector.tensor_tensor(
                            out=zq_sb[:, : gs * 128],
                            in0=a_sb[:, : gs * 128],
                            in1=pz[:, : gs * 128],
                            op=mybir.AluOpType.add,
                        )
                        s_sb = sp.tile([128, G * 128], bf, tag="s", name=f"s{w}_{g0}")
                        nc.scalar.activation(
                            out=s_sb[:, : gs * 128], in_=zq_sb[:, : gs * 128], func=SILU
                        )
                        M_sb = mp.tile([128, G * 128], bf, tag="M", name=f"M{w}_{g0}")
                        nc.vector.tensor_tensor(
                            out=M_sb[:, : gs * 128].rearrange("p (g j) -> p g j", g=gs),
                            in0=iota_sb[:, : gs * 128].rearrange(
                                "p (g j) -> p g j", g=gs
                            ),
                            in1=lrow_sb[:, t0 + g0 : t0 + g0 + gs].to_broadcast(
                                [128, gs, 128]
                            ),
                            op=mybir.AluOpType.is_equal,
                        )
                        for i in range(gs):
                            t = g0 + i
                            sl = slice(i * 128, (i + 1) * 128)
                            nc.tensor.matmul(
                                pS[:],
                                lhsT=s_sb[:, sl],
                                rhs=M_sb[:, sl],
                                start=(t == 0),
                                stop=(t == T - 1),
                            )
                        g0 += gs
                    sT_sb = op.tile([128, 128], bf, tag="sT", name=f"sT{w}")
                    nc.scalar.copy(out=sT_sb[:], in_=pS[:])
                    pO = Op.tile([128, 128], f32, tag="pO", name=f"pO{w}")
                    nc.tensor.matmul(
                        pO[:], lhsT=W2_sb[:], rhs=sT_sb[:], start=True, stop=False
                    )
                    nc.tensor.matmul(
                        pO[:],
                        lhsT=b2_sb[:],
                        rhs=degT_sb[:, w * 128 : (w + 1) * 128],
                        start=False,
                        stop=True,
                    )
                    o_sb = op.tile([128, 128], f32, tag="o", name=f"o{w}")
                    nc.vector.tensor_copy(out=o_sb[:], in_=pO[:])
                    nc.sync.dma_start(out=outT[:, w * 128 : (w + 1) * 128], in_=o_sb[:])
                    t0 += T
    nc.finalize()
    return nc


def kernel(h, edge_index, bond_features, W1, b1, W2, b2):
    global LAST_EXEC_NS
    meta, in_maps = host_prep(h, edge_index, bond_features, W1, b1, W2, b2, NPC)
    nc = build(meta)

    from concourse.bass_utils import run_bass_kernel_spmd

    trace = os.environ.get("GNN_KERNEL_TRACE", "0") == "1"
    if trace:
        _install_ntff_hook()
    res = run_bass_kernel_spmd(nc, in_maps, list(range(C)), trace=trace)
    LAST_EXEC_NS = res.exec_time_ns

    outs = []
    for c in range(C):
        o = np.asarray(res.results[c]["outT"], dtype=np.float32)
        outs.append(o.T[:NPC])
    return np.ascontiguousarray(np.concatenate(outs, axis=0))


# revision 19
# speedup vs baseline: 1.0028x; 1.0028x over previous
"""TRN2 Bass kernel for ChemicalConvWithBonds GNN message passing.

h_out = segment_sum(silu([h[row] | h[col] | bond] @ W1 + b1) @ W2 + b2, row)

Strategy (8 NeuronCores, SPMD):
  - Edges are owner-sharded by destination node (row): core c owns nodes
    [c*6250, (c+1)*6250). The host sorts edges by row and pads each
    128-node window to a tile count shared across all cores (one SPMD graph).
  - Device precomputes P = h@W1a + b1 and Q = h@W1b for its node slice
    (bf16), then AllGathers Q so every core holds the full Q table.
  - Per 128-edge tile: Q[col] rows fetched by indirect DMA; P[row] applied
    with a one-hot matmul from the window's P block (no gather); bond@W1c
    by matmul; all summed in PSUM; SiLU on the scalar engine; scatter-add
    into S^T via a one-hot matmul; per window O^T = W2-matmul + b2 x deg.
"""

import os
import sys

for p in ("/opt/trn_rl_repo", "/root/.axon_site/_ro/trn_rl_repo"):
    if os.path.isdir(p) and p not in sys.path:
        sys.path.append(p)

import contextlib
import ctypes
import types

import numpy as np
import ml_dtypes

import concourse.tile as tile
from concourse import mybir
from concourse.bacc import Bacc
from concourse.bass import IndirectOffsetOnAxis
from concourse.tile_rust import add_dep_helper

C = 8
H = 128
BOND = 16
G = 4
NPC = 6250
SCRATCH = 57344

bf = mybir.dt.bfloat16
f32 = mybir.dt.float32
i32 = mybir.dt.int32
nbf = ml_dtypes.bfloat16

LAST_EXEC_NS = None


def _install_ntff_hook():
    """Provide antenv.axon_hooks so trace=True works under axon."""
    if "antenv.axon_hooks" in sys.modules:
        return
    so_path = "/opt/axon/libaxon_pjrt.so"
    try:
        lib = ctypes.CDLL(so_path)
        lib.axon_start_nrt_profile.argtypes = [
            ctypes.POINTER(ctypes.c_int64),
            ctypes.c_size_t,
        ]
        lib.axon_start_nrt_profile.restype = ctypes.c_int64
        lib.axon_stop_nrt_profile.argtypes = [ctypes.c_char_p]
        lib.axon_stop_nrt_profile.restype = ctypes.c_int64
    except Exception:
        return

    @contextlib.contextmanager
    def _hook(output_dir, device_ids):
        import jax

        jax.devices()
        if device_ids:
            ids = (ctypes.c_int64 * len(device_ids))(*device_ids)
            rc = lib.axon_start_nrt_profile(ids, len(device_ids))
        else:
            rc = lib.axon_start_nrt_profile(None, 0)
        if rc != 0:
            raise RuntimeError(f"axon_start_nrt_profile rc={rc}")
        try:
            yield
        finally:
            n = lib.axon_stop_nrt_profile(str(output_dir).encode())
            print(f"profile: {n} file(s) -> {output_dir}", file=sys.stderr)

    mod = types.ModuleType("antenv.axon_hooks")
    mod.get_axon_ntff_profile_hook = lambda: _hook
    mod.set_axon_ntff_profile_hook = lambda h: None
    sys.modules["antenv.axon_hooks"] = mod


def host_prep(h, edge_index, bond_features, W1, b1, W2, b2, npc=NPC):
    N, _ = h.shape
    W = (npc + 127) // 128
    npad = W * 128
    nhalf = (C // 2) * npad  # Q table split point (int16 index limit)
    QSUB = 1024  # edges per dma_gather call

    row = np.asarray(edge_index[0], dtype=np.int64)
    col = np.asarray(edge_index[1], dtype=np.int64)
    perm = np.argsort(row, kind="stable")
    rs = row[perm]
    cs = col[perm]
    bs = np.asarray(bond_features, dtype=np.float32)[perm]
    qp = (cs // npc) * npad + cs % npc  # padded Q row index

    cb = np.searchsorted(rs, np.arange(C + 1) * npc)
    wb = np.zeros((C, W + 1), dtype=np.int64)
    for c in range(C):
        lo, hi = cb[c], cb[c + 1]
        wb[c] = lo + np.searchsorted(rs[lo:hi], c * npc + np.arange(W + 1) * 128)

    # per (core, window): split by Q half; pad each half to shared tile counts
    cntA = np.zeros((C, W), np.int64)
    cntB = np.zeros((C, W), np.int64)
    for c in range(C):
        for w in range(W):
            lo, hi = wb[c, w], wb[c, w + 1]
            hv = qp[lo:hi] >= nhalf
            cntB[c, w] = hv.sum()
            cntA[c, w] = (hi - lo) - cntB[c, w]
    NIa = (-(-cntA.max(axis=0) // 128) * 128).astype(np.int64)
    NIb = (-(-cntB.max(axis=0) // 128) * 128).astype(np.int64)
    Tw = ((NIa + NIb) // 128).astype(np.int64)
    Tw = np.maximum(Tw, 1)
    NIa = np.where(NIa + NIb == 0, 128, NIa)
    NT = int(Tw.sum())
    t0s = np.concatenate([[0], np.cumsum(Tw)])

    lrow = np.full((C, NT * 128), 255.0, np.float32)
    bondT = np.zeros((C, BOND, NT * 128), np.float32)
    degT = np.zeros((C, 1, npad), np.float32)
    q16 = np.zeros((C, 128, NT * 8), np.int16)  # wrapped+replicated gather idx
    p16 = np.zeros((C, 128, NT * 8), np.int16)

    def wrap16(dst, base_col, vals):
        # idx i of this call at [i%16 (+16g), base_col + i//16]
        n = len(vals)
        blk = vals.reshape(n // 16, 16).T.astype(np.int16)
        for g in range(8):
            dst[g * 16 : (g + 1) * 16, base_col : base_col + n // 16] = blk

    for c in range(C):
        deg = np.bincount(rs[cb[c] : cb[c + 1]] - c * npc, minlength=npad)
        degT[c, 0, :] = deg[:npad]
        for w in range(W):
            lo, hi = wb[c, w], wb[c, w + 1]
            qv = qp[lo:hi]
            hv = qv >= nhalf
            order = np.argsort(hv, kind="stable")
            na, nb = int(cntA[c, w]), int(cntB[c, w])
            s = int(t0s[w]) * 128
            # half A occupies slots [0, NIa), half B [NIa, NIa+NIb)
            posA = s + np.arange(na)
            posB = s + int(NIa[w]) + np.arange(nb)
            pos = np.concatenate([posA, posB])
            src_idx = lo + order
            lrow[c][pos] = rs[src_idx] - c * npc - w * 128
            bondT[c][:, pos] = bs[src_idx].T
            qa = np.zeros(int(NIa[w]), np.int64)
            qa[:na] = qv[order[:na]]
            qb = np.zeros(int(NIb[w]), np.int64)
            qb[:nb] = qv[order[na:]] - nhalf
            # wrap per QSUB-sized call
            for half, arr in ((0, qa), (1, qb)):
                off = 0
                base = s if half == 0 else s + int(NIa[w])
                while off < len(arr):
                    ni = min(QSUB, len(arr) - off)
                    wrap16(q16[c], (base + off) // 16, arr[off : off + ni])
                    off += ni
            # P gather indices: local row per slot (pads -> 0)
            pv = np.zeros(int(Tw[w]) * 128, np.int64)
            pv[pos - s] = rs[src_idx] - c * npc
            off = 0
            while off < len(pv):
                ni = min(QSUB, len(pv) - off)
                wrap16(p16[c], (s + off) // 16, pv[off : off + ni])
                off += ni

    W1 = np.asarray(W1, np.float32)
    W1ab = np.ascontiguousarray(np.concatenate([W1[:H], W1[H : 2 * H]], axis=1))
    W1cT = np.asarray(W1[2 * H :], np.float32).astype(nbf)
    b1z = np.concatenate([np.asarray(b1, np.float32), np.zeros(H, np.float32)])[None]
    hT_all = np.ascontiguousarray(np.asarray(h, np.float32).T)

    shared = {
        "W1ab": W1ab,
        "b1z": b1z,
        "onesr": np.ones((1, H), np.float32),
        "W1cT": W1cT,
        "W2p": np.asarray(W2, np.float32).astype(nbf),
        "b2r": np.asarray(b2, np.float32)[None].astype(nbf),
        "ident": np.eye(H, dtype=np.float32).astype(nbf),
        "iotag": np.tile(np.arange(128, dtype=np.float32), (128, G)).astype(nbf),
    }
    in_maps = []
    for c in range(C):
        hT_c = np.zeros((H, npad), np.float32)
        hT_c[:, :npc] = hT_all[:, c * npc : (c + 1) * npc]
        m = dict(shared)
        m["hT"] = hT_c
        m["q16"] = q16[c]
        m["p16"] = p16[c]
        m["lrowp"] = np.ascontiguousarray(
            lrow[c].reshape(NT, 128).T
        ).astype(nbf)
        m["bondT"] = bondT[c].astype(nbf)
        m["degT"] = degT[c].astype(nbf)
        in_maps.append(m)

    meta = {
        "npc": npc,
        "npad": npad,
        "W": W,
        "Tw": [int(x) for x in Tw],
        "NIa": [int(x) for x in NIa],
        "NIb": [int(x) for x in NIb],
        "NT": NT,
        "QSUB": QSUB,
    }
    return meta, in_maps


def build(meta):
    npad = meta["npad"]
    W = meta["W"]
    Tw = meta["Tw"]
    NIa = meta["NIa"]
    NIb = meta["NIb"]
    NT = meta["NT"]
    QSUB = meta["QSUB"]
    Twmax = max(Tw)
    nhalf = (C // 2) * npad

    nc = Bacc(dynamic_dma_scratch_size=SCRATCH, num_swdge_queues=4)
    hT = nc.declare_dram_parameter("hT", [H, npad], f32, isOutput=False)
    W1ab = nc.declare_dram_parameter("W1ab", [H, 2 * H], f32, isOutput=False)
    b1z = nc.declare_dram_parameter("b1z", [1, 2 * H], f32, isOutput=False)
    onesr = nc.declare_dram_parameter("onesr", [1, H], f32, isOutput=False)
    W1cT = nc.declare_dram_parameter("W1cT", [BOND, H], bf, isOutput=False)
    W2p = nc.declare_dram_parameter("W2p", [H, H], bf, isOutput=False)
    b2r = nc.declare_dram_parameter("b2r", [1, H], bf, isOutput=False)
    ident = nc.declare_dram_parameter("ident", [H, H], bf, isOutput=False)
    iotag = nc.declare_dram_parameter("iotag", [128, G * 128], bf, isOutput=False)
    q16 = nc.declare_dram_parameter("q16", [128, NT * 8], mybir.dt.int16, isOutput=False)
    p16 = nc.declare_dram_parameter("p16", [128, NT * 8], mybir.dt.int16, isOutput=False)
    lrowp = nc.declare_dram_parameter("lrowp", [128, NT], bf, isOutput=False)
    bondT = nc.declare_dram_parameter("bondT", [BOND, NT * 128], bf, isOutput=False)
    degT = nc.declare_dram_parameter("degT", [1, npad], bf, isOutput=False)
    outT = nc.declare_dram_parameter("outT", [H, npad], f32, isOutput=True)

    P_loc = nc.dram_tensor("P_loc", [npad, H], bf)
    Q_self = nc.dram_tensor("Q_self", [npad, H], bf)
    Q_full = nc.dram_tensor("Q_full", [C * npad, H], bf, addr_space="Shared")

    SILU = mybir.ActivationFunctionType.Silu

    with tile.TileContext(nc) as tc:
        with tc.tile_pool(name="cst", bufs=1) as cp:
            W1ab_sb = cp.tile([H, 2 * H], f32)
            nc.sync.dma_start(out=W1ab_sb[:], in_=W1ab[:])
            b1z_sb = cp.tile([1, 2 * H], f32)
            nc.sync.dma_start(out=b1z_sb[:], in_=b1z[:])
            ones_sb = cp.tile([1, H], f32)
            nc.sync.dma_start(out=ones_sb[:], in_=onesr[:])
            W1c_sb = cp.tile([BOND, H], bf)
            nc.sync.dma_start(out=W1c_sb[:], in_=W1cT[:])
            W2_sb = cp.tile([H, H], bf)
            nc.sync.dma_start(out=W2_sb[:], in_=W2p[:])
            b2_sb = cp.tile([1, H], bf)
            nc.sync.dma_start(out=b2_sb[:], in_=b2r[:])
            id_sb = cp.tile([H, H], bf)
            nc.sync.dma_start(out=id_sb[:], in_=ident[:])
            iota_sb = cp.tile([128, G * 128], bf)
            nc.sync.dma_start(out=iota_sb[:], in_=iotag[:])
            q16_sb = cp.tile([128, NT * 8], mybir.dt.int16)
            q16_ld = nc.sync.dma_start(out=q16_sb[:], in_=q16[:])
            p16_sb = cp.tile([128, NT * 8], mybir.dt.int16)
            p16_ld = nc.sync.dma_start(out=p16_sb[:], in_=p16[:])
            lrow_sb = cp.tile([128, NT], bf)
            nc.sync.dma_start(out=lrow_sb[:], in_=lrowp[:])
            degT_sb = cp.tile([1, npad], bf)
            nc.sync.dma_start(out=degT_sb[:], in_=degT[:])

            # ---- Phase A ----
            p_writes = []
            pa_ctx = tc.tile_pool(name="pa", bufs=2)
            pap_ctx = tc.tile_pool(name="pap", bufs=2, space="PSUM")
            pa = pa_ctx.__enter__()
            pap = pap_ctx.__enter__()
            hT_sb = pa.tile([H, npad], f32, tag="hTsb", name="hTsb")
            nc.sync.dma_start(out=hT_sb[:], in_=hT[:])
            for w in range(W):
                ppq = pap.tile([128, 2 * H], f32, tag="ppq", name=f"ppq{w}")
                nc.tensor.matmul(
                    ppq[:],
                    lhsT=hT_sb[:, w * 128 : (w + 1) * 128],
                    rhs=W1ab_sb[:],
                    start=True,
                    stop=False,
                )
                nc.tensor.matmul(
                    ppq[:], lhsT=ones_sb[:], rhs=b1z_sb[:], start=False, stop=True
                )
                pq_sb = pa.tile([128, 2 * H], bf, tag="pqsb", name=f"pq{w}")
                nc.scalar.copy(out=pq_sb[:], in_=ppq[:])
                p_writes.append(
                    nc.sync.dma_start(
                        out=P_loc[w * 128 : (w + 1) * 128, :], in_=pq_sb[:, 0:H]
                    )
                )
                nc.sync.dma_start(
                    out=Q_self[w * 128 : (w + 1) * 128, :], in_=pq_sb[:, H : 2 * H]
                )

            cc = nc.gpsimd.collective_compute(
                "AllGather",
                mybir.AluOpType.bypass,
                replica_groups=[list(range(C))],
                ins=[Q_self[:]],
                outs=[Q_full[:]],
            )
            pap_ctx.__exit__(None, None, None)
            pa_ctx.__exit__(None, None, None)

            # ---- Phase B ----
            qnum = 0
            with (
                tc.tile_pool(name="gp", bufs=3) as gp,
                tc.tile_pool(name="bp", bufs=2) as bp,
                tc.tile_pool(name="sp", bufs=4) as sp,
                tc.tile_pool(name="mp", bufs=4) as mp,
                tc.tile_pool(name="zp", bufs=3, space="PSUM") as zp,
                tc.tile_pool(name="Sp", bufs=2, space="PSUM") as Sp,
                tc.tile_pool(name="Op", bufs=1, space="PSUM") as Op,
                tc.tile_pool(name="op", bufs=3) as op,
            ):
                t0 = 0
                for w in range(W):
                    T = Tw[w]
                    bT = bp.tile([BOND, T * 128], bf, tag="bT", name=f"bT{w}")
                    nc.sync.dma_start(
                        out=bT[:], in_=bondT[:, t0 * 128 : (t0 + T) * 128]
                    )
                    # ---- P gathers (local table), rotating queues
                    Pg = gp.tile([128, T * 128], bf, tag="Pg", name=f"Pg{w}")
                    off = 0
                    while off < T * 128:
                        ni = min(QSUB, T * 128 - off)
                        gi = nc.gpsimd.dma_gather(
                            out_ap=Pg[:, off : off + ni].rearrange(
                                "p (k h) -> p k h", h=H
                            ),
                            in_ap=P_loc[:],
                            idxs_ap=p16_sb[
                                :, (t0 * 128 + off) // 16 : (t0 * 128 + off + ni) // 16
                            ],
                            num_idxs=ni,
                            num_idxs_reg=ni,
                            elem_size=H,
                            queue_num=qnum % 4,
                        )
                        qnum += 1
                        add_dep_helper(gi.ins, p_writes[w].ins, sync=True, reason="P win ready")
                        add_dep_helper(gi.ins, p16_ld.ins, sync=True, reason="after idx")
                        off += ni
                    # ---- Q gathers: per half, per QSUB chunk, rotating queues
                    Qg = gp.tile([128, T * 128], bf, tag="Qg", name=f"Qg{w}")
                    for half in range(2):
                        ni_h = NIa[w] if half == 0 else NIb[w]
                        base = 0 if half == 0 else NIa[w]
                        rowbase = 0 if half == 0 else nhalf
                        off = 0
                        while off < ni_h:
                            ni = min(QSUB, ni_h - off)
                            slot = base + off
                            gi = nc.gpsimd.dma_gather(
                                out_ap=Qg[:, slot : slot + ni].rearrange(
                                    "p (k h) -> p k h", h=H
                                ),
                                in_ap=Q_full[rowbase : rowbase + nhalf, :],
                                idxs_ap=q16_sb[
                                    :, (t0 * 128 + slot) // 16 : (t0 * 128 + slot + ni) // 16
                                ],
                                num_idxs=ni,
                                num_idxs_reg=ni,
                                elem_size=H,
                                queue_num=qnum % 4,
                            )
                            qnum += 1
                            add_dep_helper(
                                gi.ins, cc.ins, sync=True, reason="after AllGather"
                            )
                            add_dep_helper(
                                gi.ins, q16_ld.ins, sync=True, reason="after idx"
                            )
                            off += ni
                    pS = Sp.tile([128, 128], f32, tag="pS", name=f"pS{w}")
                    g0 = 0
                    while g0 < T:
                        gs = min(G, T - g0)
                        pz = zp.tile([128, G * 128], f32, tag="pz", name=f"pz{w}_{g0}")
                        for i in range(gs):
                            t = g0 + i
                            sl = slice(i * 128, (i + 1) * 128)
                            esl = slice(t * 128, (t + 1) * 128)
                            nc.tensor.matmul(
                                pz[:, sl],
                                lhsT=bT[:, esl],
                                rhs=W1c_sb[:],
                                start=True,
                                stop=True,
                            )
                        a_sb = sp.tile([128, G * 128], bf, tag="a", name=f"a{w}_{g0}")
                        nc.vector.tensor_tensor(
                            out=a_sb[:, : gs * 128],
                            in0=Pg[:, g0 * 128 : g0 * 128 + gs * 128],
                            in1=Qg[:, g0 * 128 : g0 * 128 + gs * 128],
                            op=mybir.AluOpType.add,
                        )
                        zq_sb = sp.tile(
                            [128, G * 128], f32, tag="zq", name=f"zq{w}_{g0}"
                        )
                        nc.vector.tensor_tensor(
                            out=zq_sb[:, : gs * 128],
                            in0=a_sb[:, : gs * 128],
                            in1=pz[:, : gs * 128],
                            op=mybir.AluOpType.add,
                        )
                        s_sb = sp.tile([128, G * 128], bf, tag="s", name=f"s{w}_{g0}")
                        nc.scalar.activation(
                            out=s_sb[:, : gs * 128], in_=zq_sb[:, : gs * 128], func=SILU
                        )
                        M_sb = mp.tile([128, G * 128], bf, tag="M", name=f"M{w}_{g0}")
                        nc.vector.tensor_tensor(
                            out=M_sb[:, : gs * 128].rearrange("p (g j) -> p g j", g=gs),
                            in0=iota_sb[:, : gs * 128].rearrange(
                                "p (g j) -> p g j", g=gs
                            ),
                            in1=lrow_sb[:, t0 + g0 : t0 + g0 + gs].to_broadcast(
                                [128, gs, 128]
                            ),
                            op=mybir.AluOpType.is_equal,
                        )
                        for i in range(gs):
                            t = g0 + i
                            sl = slice(i * 128, (i + 1) * 128)
                            nc.tensor.matmul(
                                pS[:],
                                lhsT=s_sb[:, sl],
                                rhs=M_sb[:, sl],
                                start=(t == 0),
                                stop=(t == T - 1),
                            )
                        g0 += gs
                    sT_sb = op.tile([128, 128], bf, tag="sT", name=f"sT{w}")
                    nc.scalar.copy(out=sT_sb[:], in_=pS[:])
                    pO = Op.tile([128, 128], f32, tag="pO", name=f"pO{w}")
                    nc.tensor.matmul(
                        pO[:], lhsT=W2_sb[:], rhs=sT_sb[:], start=True, stop=False
                    )
                    nc.tensor.matmul(
                        pO[:],
                        lhsT=b2_sb[:],
                        rhs=degT_sb[:, w * 128 : (w + 1) * 128],
                        start=False,
                        stop=True,
                    )
                    o_sb = op.tile([128, 128], f32, tag="o", name=f"o{w}")
                    nc.vector.tensor_copy(out=o_sb[:], in_=pO[:])
                    nc.sync.dma_start(out=outT[:, w * 128 : (w + 1) * 128], in_=o_sb[:])
                    t0 += T
    nc.finalize()
    return nc


def kernel(h, edge_index, bond_features, W1, b1, W2, b2):
    global LAST_EXEC_NS
    meta, in_maps = host_prep(h, edge_index, bond_features, W1, b1, W2, b2, NPC)
    nc = build(meta)

    from concourse.bass_utils import run_bass_kernel_spmd

    trace = os.environ.get("GNN_KERNEL_TRACE", "0") == "1"
    if trace:
        _install_ntff_hook()
    res = run_bass_kernel_spmd(nc, in_maps, list(range(C)), trace=trace)
    LAST_EXEC_NS = res.exec_time_ns

    outs = []
    for c in range(C):
        o = np.asarray(res.results[c]["outT"], dtype=np.float32)
        outs.append(o.T[:NPC])
    return np.ascontiguousarray(np.concatenate(outs, axis=0))
